# revision 28
# baseline (speedup 1.0000x reference)
"""Additive attention (Bahdanau) Trainium2 kernel, SPMD over 8 NeuronCores.

Reference computation (per batch b):
    q = queries @ W_q                    [Q, H]
    k = keys    @ W_k                    [K, H]
    scores[q,k] = sum_h w_v[h] * tanh(q[q,h] + k[k,h])
    attn = masked_softmax(scores, valid_len)   (keys >= valid_len masked out)
    out = attn @ values                  [Q, Dv]

Distribution: queries are sharded across the 8 cores (each core gets a
32-query slice of every batch); keys/values/weights are replicated. Each
core computes the same-shaped, perfectly load-balanced slice of the
output. Keys are truncated per batch to the valid length (rounded up to
an even count), which prunes the dominant tanh work for every core
equally. No collectives needed.

Device algorithm per (core, batch):
  - kprojT [h=128, n] = W_k^T @ keys^T      (keys pre-transposed on host)
  - qprojT [h=128, 32] = W_q^T @ q-slice^T
  - x = kprojT + qprojT[:, qi]   (tensor_scalar add, bf16, DVE/GPSIMD)
  - f = tanh(x)                  (ScalarE, fused over 8 queries per op)
  - scoresT[k, qi] = f_chunk^T @ w_v   (PE, N=1 matmuls into PSUM columns)
  - pT = exp(scoresT [+ mask bias on the boundary chunk])
        no max-subtraction needed: |scores| <= ||w_v||_1 ~ 9, exp can't
        overflow fp32; masked lanes get bias -30000 -> exp == 0 exactly.
  - out[qi, v] = (pT^T @ [V | 1]) , then divide by the appended ones-column
    row-sum (softmax normalization).
"""

import numpy as np
import ml_dtypes

import concourse.bass as bass
import concourse.tile as tile
import concourse.bacc as bacc
from concourse import mybir
from concourse.bass_utils import run_bass_kernel_spmd

BF16 = mybir.dt.bfloat16
F32 = mybir.dt.float32
TANH = mybir.ActivationFunctionType.Tanh
EXP = mybir.ActivationFunctionType.Exp

B, Q, K, D, H, DV = 8, 256, 1024, 256, 128, 128
NCORES = 8
QSH = Q // NCORES          # queries per core per batch
GQ = 16                    # queries fused per tanh op
NEG_BIAS = -30000.0        # exp(-30000) == 0.0 exactly in fp32

_graph_cache: dict = {}


def _npad(vl):
    """Per-batch key extent: valid length rounded up to even (DVE 4x mode)."""
    return int(min(max(2 * ((vl + 1) // 2), 2), K))


def _build(nps):
    """Build the SPMD graph. nps: tuple of per-batch (even) key extents."""
    nc = bacc.Bacc("TRN2", target_bir_lowering=False, debug=False,
                   num_devices=NCORES)
    KT = sum(nps)
    kcs = [(n + 127) // 128 for n in nps]
    # batch emission order: small first (fast pipeline start), small last
    # (short epilogue tail), big ones in the middle.
    # two smallish batches first (fast pipeline start while big DMAs
    # land), then the rest largest-first, ending on the smallest batch
    # (short epilogue tail).
    asc = sorted(range(B), key=lambda b: kcs[b])
    order = [asc[1], asc[2]] + \
        sorted(asc[3:], key=lambda b: -kcs[b]) + [asc[0]]

    kT_d = nc.dram_tensor("kT", (D, KT), F32, kind="ExternalInput").ap()
    v_d = nc.dram_tensor("vals", (KT, DV + 2), BF16, kind="ExternalInput").ap()
    qT_d = nc.dram_tensor("qT", (D, B * QSH), F32, kind="ExternalInput").ap()
    wq_d = nc.dram_tensor("wq", (D, H), F32, kind="ExternalInput").ap()
    wk_d = nc.dram_tensor("wk", (D, H), F32, kind="ExternalInput").ap()
    wv_d = nc.dram_tensor("wv", (H, 1), BF16, kind="ExternalInput").ap()
    bias_d = nc.dram_tensor("biasT", (128, B), F32, kind="ExternalInput").ap()
    out_d = nc.dram_tensor("out", (B, QSH, DV), F32, kind="ExternalOutput").ap()

    offs = np.concatenate([[0], np.cumsum(nps)]).astype(int)

    with tile.TileContext(nc) as tc:
        with (
            tc.tile_pool(name="const", bufs=1) as const,
            tc.tile_pool(name="kt", bufs=4) as kt_pool,
            tc.tile_pool(name="kproj", bufs=4) as kproj_pool,
            tc.tile_pool(name="qproj", bufs=4) as qproj_pool,
            tc.tile_pool(name="vt", bufs=sum(kcs)) as vpool,
            tc.tile_pool(name="x", bufs=3) as xpool,
            tc.tile_pool(name="pT", bufs=2) as ppool,
            tc.tile_pool(name="osb", bufs=2) as osb_pool,
            tc.tile_pool(name="proj_ps", bufs=3, space="PSUM") as proj_ps,
            tc.tile_pool(name="sc_ps", bufs=3, space="PSUM") as sc_ps_pool,
            tc.tile_pool(name="out_ps", bufs=2, space="PSUM") as out_ps_pool,
        ):
            # ---- constants + first-batch DMA (gpsimd queue starts first) --
            b0 = order[0]
            dmad = {}

            def dma_a(b, first=False):
                n, off = nps[b], offs[b]
                eng = nc.gpsimd if first else nc.sync
                kT0 = kt_pool.tile([128, n], F32, tag="kT0")
                kT1 = kt_pool.tile([128, n], F32, tag="kT1")
                eng.dma_start(kT0[:], kT_d[0:128, off:off + n])
                eng.dma_start(kT1[:], kT_d[128:256, off:off + n])
                qt = kt_pool.tile([128, 2, QSH], F32, tag="qt")
                eng.dma_start(qt[:, 0, :], qT_d[0:128, b * QSH:(b + 1) * QSH])
                eng.dma_start(qt[:, 1, :], qT_d[128:256, b * QSH:(b + 1) * QSH])
                vt_b = []
                for c in range((n + 127) // 128):
                    m = min(128, n - c * 128)
                    vt = vpool.tile([128, DV + 2], BF16, tag="vt")
                    r0 = off + c * 128
                    nc.gpsimd.dma_start(vt[:m, :], v_d[r0:r0 + m, :])
                    vt_b.append((vt, m))
                dmad[b] = (kT0, kT1, qt, vt_b)

            dma_a(b0, first=True)

            wq_sb = const.tile([128, 2, H], F32, tag="wq")
            nc.sync.dma_start(wq_sb[:, 0, :], wq_d[0:128, :])
            nc.sync.dma_start(wq_sb[:, 1, :], wq_d[128:256, :])
            wk_sb = const.tile([128, 2, H], F32, tag="wk")
            nc.sync.dma_start(wk_sb[:, 0, :], wk_d[0:128, :])
            nc.sync.dma_start(wk_sb[:, 1, :], wk_d[128:256, :])
            wv_sb = const.tile([H, 1], BF16, tag="wv")
            nc.sync.dma_start(wv_sb[:], wv_d[:, :])
            bias_sb = const.tile([128, B], F32, tag="bias")
            nc.sync.dma_start(bias_sb[:], bias_d[:, :])

            projd = {}

            def proj_a(b):
                n = nps[b]
                kT0, kT1, qt, vt_b = dmad.pop(b)
                kp = kproj_pool.tile([128, n], BF16, tag="kp")
                for j0 in range(0, n, 512):
                    w = min(512, n - j0)
                    ps = proj_ps.tile([128, w], F32, tag="ps")
                    nc.tensor.matmul(ps[:], wk_sb[:, 0, :], kT0[:, j0:j0 + w],
                                     start=True, stop=False)
                    nc.tensor.matmul(ps[:], wk_sb[:, 1, :], kT1[:, j0:j0 + w],
                                     start=False, stop=True)
                    nc.vector.tensor_copy(kp[:, j0:j0 + w], ps[:])
                qp = qproj_pool.tile([128, QSH], F32, tag="qp")
                ps = proj_ps.tile([128, QSH], F32, tag="ps")
                nc.tensor.matmul(ps[:], wq_sb[:, 0, :], qt[:, 0, :],
                                 start=True, stop=False)
                nc.tensor.matmul(ps[:], wq_sb[:, 1, :], qt[:, 1, :],
                                 start=False, stop=True)
                nc.vector.tensor_copy(qp[:], ps[:])
                projd[b] = (kp, qp, vt_b)

            for bb in order[1:4]:
                dma_a(bb)
            proj_a(b0)

            # ---- per-batch pipeline ----
            # DMAs issued 2 batches ahead; projections + fp32->bf16 casts
            # 1 batch ahead (so the in-order DVE never stalls on a DMA);
            # exp+final-matmul of batch i-1 fire after batch i's first
            # score group; reciprocal/divide of batch i-1 (DVE, waits on
            # PE's final matmul) fire only after ALL of batch i's adds.
            pend_exp = None
            pend_div = None
            for bi, b in enumerate(order):
                n = nps[b]
                kcb = kcs[b]
                m_last = n - (kcb - 1) * 128
                kprojT_b, qprojT_b, vt_b = projd.pop(b)
                if bi + 1 < B:
                    proj_a(order[bi + 1])
                sc = sc_ps_pool.tile([128, kcb * QSH], F32, tag="sc")
                if m_last < 128:
                    # kill stale PSUM rows in the partial chunk so
                    # exp(stale + bias) can't produce inf/nan; partition
                    # base must be 32-aligned, matmuls rewrite [0,m_last)
                    m0 = (m_last // 32) * 32
                    for p0 in range(m0, 128, 32):
                        nc.vector.memset(sc[p0:p0 + 32, (kcb - 1) * QSH:], 0.0)
                for g in range(QSH // GQ):
                    x = xpool.tile([128, GQ * n], BF16, tag="x")
                    for j in range(GQ):
                        qi = g * GQ + j
                        nc.vector.tensor_scalar_add(
                            x[:, j * n:(j + 1) * n], kprojT_b[:],
                            qprojT_b[:, qi:qi + 1])
                    nc.scalar.activation(x[:], x[:], TANH)  # in-place
                    for j in range(GQ):
                        qi = g * GQ + j
                        for c in range(kcb):
                            m = min(128, n - c * 128)
                            col = c * QSH + qi
                            nc.tensor.matmul(
                                sc[:m, col:col + 1],
                                x[:, j * n + c * 128:j * n + c * 128 + m],
                                wv_sb[:],
                                start=True, stop=True)
                    if g == 0:
                        if pend_exp is not None:
                            pend_exp()
                            pend_exp = None
                        if bi + 4 < B:
                            dma_a(order[bi + 4])
                if pend_div is not None:
                    pend_div()
                    pend_div = None

                state = {}

                def make_exp_final(b=b, kcb=kcb, sc=sc, vt_b=vt_b,
                                   state=state):
                    def exp_final():
                        pT = ppool.tile([128, kcb * QSH], BF16, tag="pT")
                        last0 = (kcb - 1) * QSH
                        if kcb > 1:
                            nc.scalar.activation(pT[:, 0:last0],
                                                 sc[:, 0:last0], EXP)
                        nc.scalar.activation(pT[:, last0:], sc[:, last0:],
                                             EXP, bias=bias_sb[:, b:b + 1])
                        ops = out_ps_pool.tile([QSH, DV + 1], F32, tag="ops")
                        for c in range(kcb):
                            vt, m = vt_b[c]
                            nc.tensor.matmul(ops[:],
                                             pT[:m, c * QSH:(c + 1) * QSH],
                                             vt[:m, 0:DV + 1],
                                             start=(c == 0),
                                             stop=(c == kcb - 1))
                        state["ops"] = ops
                    return exp_final

                def make_div(b=b, state=state):
                    def div():
                        ops = state["ops"]
                        r = osb_pool.tile([QSH, 1], F32, tag="r")
                        nc.vector.reciprocal(r[:], ops[:, DV:DV + 1])
                        osb = osb_pool.tile([QSH, DV], F32, tag="osb")
                        nc.vector.tensor_scalar_mul(osb[:], ops[:, 0:DV], r[:])
                        nc.sync.dma_start(out_d[b, :, :], osb[:])
                    return div

                pend_exp = make_exp_final()
                pend_div = make_div()
            pend_exp()
            pend_div()
    nc.compile()
    return nc


def _prep(queries, keys, values, valid_lens):
    vl = np.asarray(valid_lens).astype(np.int64)
    nps = tuple(_npad(int(l)) for l in vl)
    KT = sum(nps)

    kT = np.empty((D, KT), np.float32)
    vals = np.zeros((KT, DV + 2), ml_dtypes.bfloat16)
    biasT = np.zeros((128, B), np.float32)
    off = 0
    for b in range(B):
        n = nps[b]
        kT[:, off:off + n] = keys[b, :n, :].T
        vals[off:off + n, 0:DV] = values[b, :n, :].astype(ml_dtypes.bfloat16)
        vals[off:off + n, DV] = ml_dtypes.bfloat16(1.0)
        kcb = (n + 127) // 128
        j = np.arange(128)
        valid = (kcb - 1) * 128 + j < vl[b]
        biasT[:, b] = np.where(valid, 0.0, NEG_BIAS).astype(np.float32)
        off += n

    qT_shards = []
    for i in range(NCORES):
        qt = np.empty((D, B * QSH), np.float32)
        for b in range(B):
            qt[:, b * QSH:(b + 1) * QSH] = queries[b, i * QSH:(i + 1) * QSH, :].T
        qT_shards.append(qt)
    return nps, kT, vals, biasT, qT_shards


def run(queries, keys, values, valid_lens, W_q, W_k, w_v, **run_kwargs):
    """Full pipeline; returns (output, BassKernelResults)."""
    queries = np.asarray(queries, np.float32)
    keys = np.asarray(keys, np.float32)
    values = np.asarray(values, np.float32)
    W_q = np.asarray(W_q, np.float32)
    W_k = np.asarray(W_k, np.float32)
    w_v = np.asarray(w_v, np.float32)

    nps, kT, vals, biasT, qT_shards = _prep(queries, keys, values, valid_lens)
    wv = np.ascontiguousarray(w_v.reshape(H, 1)).astype(ml_dtypes.bfloat16)
    common = {
        "kT": np.ascontiguousarray(kT),
        "vals": np.ascontiguousarray(vals),
        "wq": np.ascontiguousarray(W_q),
        "wk": np.ascontiguousarray(W_k),
        "wv": wv,
        "biasT": np.ascontiguousarray(biasT),
    }
    in_maps = [dict(common, qT=np.ascontiguousarray(q)) for q in qT_shards]

    nc = _graph_cache.get(nps)
    if nc is None:
        nc = _build(nps)
        _graph_cache[nps] = nc
    res = run_bass_kernel_spmd(nc, in_maps, core_ids=list(range(NCORES)),
                               **run_kwargs)
    out = np.empty((B, Q, DV), np.float32)
    for i in range(NCORES):
        out[:, i * QSH:(i + 1) * QSH, :] = res.results[i]["out"]
    return out, res


def kernel(queries, keys, values, valid_lens, W_q, W_k, w_v):
    out, _ = run(queries, keys, values, valid_lens, W_q, W_k, w_v)
    return out


# revision 34
# speedup vs baseline: 1.0233x; 1.0233x over previous
"""Additive attention (Bahdanau) Trainium2 kernel, SPMD over 8 NeuronCores.

Reference computation (per batch b):
    q = queries @ W_q                    [Q, H]
    k = keys    @ W_k                    [K, H]
    scores[q,k] = sum_h w_v[h] * tanh(q[q,h] + k[k,h])
    attn = masked_softmax(scores, valid_len)   (keys >= valid_len masked out)
    out = attn @ values                  [Q, Dv]

Distribution: queries are sharded across the 8 cores (each core gets a
32-query slice of every batch); keys/values/weights are replicated. Each
core computes the same-shaped, perfectly load-balanced slice of the
output. Keys are truncated per batch to the valid length (rounded up to
an even count), which prunes the dominant tanh work for every core
equally. No collectives needed.

Device algorithm per (core, batch):
  - kprojT [h=128, n] = W_k^T @ keys^T      (keys pre-transposed on host)
  - qprojT [h=128, 32] = W_q^T @ q-slice^T
  - x = kprojT + qprojT[:, qi]   (tensor_scalar add, bf16, DVE/GPSIMD)
  - f = tanh(x)                  (ScalarE, fused over 8 queries per op)
  - scoresT[k, qi] = f_chunk^T @ w_v   (PE, N=1 matmuls into PSUM columns)
  - pT = exp(scoresT [+ mask bias on the boundary chunk])
        no max-subtraction needed: |scores| <= ||w_v||_1 ~ 9, exp can't
        overflow fp32; masked lanes get bias -30000 -> exp == 0 exactly.
  - out[qi, v] = (pT^T @ [V | 1]) , then divide by the appended ones-column
    row-sum (softmax normalization).
"""

import numpy as np
import ml_dtypes

import concourse.bass as bass
import concourse.tile as tile
import concourse.bacc as bacc
from concourse import mybir
from concourse.bass_utils import run_bass_kernel_spmd

BF16 = mybir.dt.bfloat16
F32 = mybir.dt.float32
TANH = mybir.ActivationFunctionType.Tanh
EXP = mybir.ActivationFunctionType.Exp

B, Q, K, D, H, DV = 8, 256, 1024, 256, 128, 128
NCORES = 8
QSH = Q // NCORES          # queries per core per batch
GQ = 16                    # queries fused per tanh op
NEG_BIAS = -30000.0        # exp(-30000) == 0.0 exactly in fp32

_graph_cache: dict = {}


def _npad(vl):
    """Per-batch key extent: valid length rounded up to even (DVE 4x mode)."""
    return int(min(max(2 * ((vl + 1) // 2), 2), K))


def _build(nps):
    """Build the SPMD graph. nps: tuple of per-batch (even) key extents."""
    nc = bacc.Bacc("TRN2", target_bir_lowering=False, debug=False,
                   num_devices=NCORES)
    KT = sum(nps)
    kcs = [(n + 127) // 128 for n in nps]
    # batch emission order: small first (fast pipeline start), small last
    # (short epilogue tail), big ones in the middle.
    # two smallish batches first (fast pipeline start while big DMAs
    # land), then the rest largest-first, ending on the smallest batch
    # (short epilogue tail).
    asc = sorted(range(B), key=lambda b: kcs[b])
    order = [asc[1], asc[2]] + \
        sorted(asc[3:], key=lambda b: -kcs[b]) + [asc[0]]

    kT_d = nc.dram_tensor("kT", (D, KT), F32, kind="ExternalInput").ap()
    v_d = nc.dram_tensor("vals", (KT, DV + 2), BF16, kind="ExternalInput").ap()
    qT_d = nc.dram_tensor("qT", (D, B * QSH), F32, kind="ExternalInput").ap()
    wq_d = nc.dram_tensor("wq", (D, H), F32, kind="ExternalInput").ap()
    wk_d = nc.dram_tensor("wk", (D, H), F32, kind="ExternalInput").ap()
    wv_d = nc.dram_tensor("wv", (H, 1), BF16, kind="ExternalInput").ap()
    bias_d = nc.dram_tensor("biasT", (128, B), F32, kind="ExternalInput").ap()
    out_d = nc.dram_tensor("out", (B, QSH, DV), F32, kind="ExternalOutput").ap()

    offs = np.concatenate([[0], np.cumsum(nps)]).astype(int)

    with tile.TileContext(nc) as tc:
        with (
            tc.tile_pool(name="const", bufs=1) as const,
            tc.tile_pool(name="kt", bufs=4) as kt_pool,
            tc.tile_pool(name="kproj", bufs=4) as kproj_pool,
            tc.tile_pool(name="qproj", bufs=4) as qproj_pool,
            tc.tile_pool(name="vt", bufs=sum(kcs)) as vpool,
            tc.tile_pool(name="x", bufs=3) as xpool,
            tc.tile_pool(name="pT", bufs=2) as ppool,
            tc.tile_pool(name="osb", bufs=2) as osb_pool,
            tc.tile_pool(name="proj_ps", bufs=3, space="PSUM") as proj_ps,
            tc.tile_pool(name="sc_ps", bufs=2, space="PSUM") as sc_ps_pool,
            tc.tile_pool(name="out_ps", bufs=3, space="PSUM") as out_ps_pool,
        ):
            # ---- constants + first-batch DMA (gpsimd queue starts first) --
            b0 = order[0]
            dmad = {}

            def dma_a(b, first=False):
                n, off = nps[b], offs[b]
                eng = nc.gpsimd if first else nc.sync
                kT0 = kt_pool.tile([128, n], F32, tag="kT0")
                kT1 = kt_pool.tile([128, n], F32, tag="kT1")
                eng.dma_start(kT0[:], kT_d[0:128, off:off + n])
                eng.dma_start(kT1[:], kT_d[128:256, off:off + n])
                qt = kt_pool.tile([128, 2, QSH], F32, tag="qt")
                eng.dma_start(qt[:, 0, :], qT_d[0:128, b * QSH:(b + 1) * QSH])
                eng.dma_start(qt[:, 1, :], qT_d[128:256, b * QSH:(b + 1) * QSH])
                vt_b = []
                for c in range((n + 127) // 128):
                    m = min(128, n - c * 128)
                    vt = vpool.tile([128, DV + 2], BF16, tag="vt")
                    r0 = off + c * 128
                    nc.gpsimd.dma_start(vt[:m, :], v_d[r0:r0 + m, :])
                    vt_b.append((vt, m))
                dmad[b] = (kT0, kT1, qt, vt_b)

            dma_a(b0, first=True)

            wq_sb = const.tile([128, 2, H], F32, tag="wq")
            nc.sync.dma_start(wq_sb[:, 0, :], wq_d[0:128, :])
            nc.sync.dma_start(wq_sb[:, 1, :], wq_d[128:256, :])
            wk_sb = const.tile([128, 2, H], F32, tag="wk")
            nc.sync.dma_start(wk_sb[:, 0, :], wk_d[0:128, :])
            nc.sync.dma_start(wk_sb[:, 1, :], wk_d[128:256, :])
            wv_sb = const.tile([H, 1], BF16, tag="wv")
            nc.sync.dma_start(wv_sb[:], wv_d[:, :])
            bias_sb = const.tile([128, B], F32, tag="bias")
            nc.sync.dma_start(bias_sb[:], bias_d[:, :])

            projd = {}

            def proj_a(b):
                n = nps[b]
                kT0, kT1, qt, vt_b = dmad.pop(b)
                kp = kproj_pool.tile([128, n], BF16, tag="kp")
                for j0 in range(0, n, 512):
                    w = min(512, n - j0)
                    ps = proj_ps.tile([128, w], F32, tag="ps")
                    nc.tensor.matmul(ps[:], wk_sb[:, 0, :], kT0[:, j0:j0 + w],
                                     start=True, stop=False)
                    nc.tensor.matmul(ps[:], wk_sb[:, 1, :], kT1[:, j0:j0 + w],
                                     start=False, stop=True)
                    nc.vector.tensor_copy(kp[:, j0:j0 + w], ps[:])
                qp = qproj_pool.tile([128, QSH], F32, tag="qp")
                ps = proj_ps.tile([128, QSH], F32, tag="ps")
                nc.tensor.matmul(ps[:], wq_sb[:, 0, :], qt[:, 0, :],
                                 start=True, stop=False)
                nc.tensor.matmul(ps[:], wq_sb[:, 1, :], qt[:, 1, :],
                                 start=False, stop=True)
                nc.vector.tensor_copy(qp[:], ps[:])
                projd[b] = (kp, qp, vt_b)

            for bb in order[1:4]:
                dma_a(bb)
            proj_a(b0)

            # ---- per-batch pipeline ----
            # DMAs issued 2 batches ahead; projections + fp32->bf16 casts
            # 1 batch ahead (so the in-order DVE never stalls on a DMA);
            # exp+final-matmul of batch i-1 fire after batch i's first
            # score group; reciprocal/divide of batch i-1 (DVE, waits on
            # PE's final matmul) fire only after ALL of batch i's adds.
            pend_exp = None
            div_q = []
            for bi, b in enumerate(order):
                n = nps[b]
                kcb = kcs[b]
                m_last = n - (kcb - 1) * 128
                kprojT_b, qprojT_b, vt_b = projd.pop(b)
                sc = sc_ps_pool.tile([128, kcb * QSH], F32, tag="sc")
                if m_last < 128:
                    # kill stale PSUM rows in the partial chunk so
                    # exp(stale + bias) can't produce inf/nan; partition
                    # base must be 32-aligned, matmuls rewrite [0,m_last)
                    m0 = (m_last // 32) * 32
                    for p0 in range(m0, 128, 32):
                        nc.vector.memset(sc[p0:p0 + 32, (kcb - 1) * QSH:], 0.0)
                for g in range(QSH // GQ):
                    x = xpool.tile([128, GQ * n], BF16, tag="x")
                    for j in range(GQ):
                        qi = g * GQ + j
                        nc.vector.tensor_scalar_add(
                            x[:, j * n:(j + 1) * n], kprojT_b[:],
                            qprojT_b[:, qi:qi + 1])
                    nc.scalar.activation(x[:], x[:], TANH)  # in-place
                    for j in range(GQ):
                        qi = g * GQ + j
                        for c in range(kcb):
                            m = min(128, n - c * 128)
                            col = c * QSH + qi
                            nc.tensor.matmul(
                                sc[:m, col:col + 1],
                                x[:, j * n + c * 128:j * n + c * 128 + m],
                                wv_sb[:],
                                start=True, stop=True)
                    if g == 0:
                        if pend_exp is not None:
                            pend_exp()
                            pend_exp = None
                        if div_q and len(div_q) >= 2:
                            div_q.pop(0)()
                        if bi + 4 < B:
                            dma_a(order[bi + 4])
                        if bi + 1 < B:
                            proj_a(order[bi + 1])
                state = {}

                def make_exp_final(b=b, kcb=kcb, sc=sc, vt_b=vt_b,
                                   state=state):
                    def exp_final():
                        pT = ppool.tile([128, kcb * QSH], BF16, tag="pT")
                        last0 = (kcb - 1) * QSH
                        if kcb > 1:
                            nc.scalar.activation(pT[:, 0:last0],
                                                 sc[:, 0:last0], EXP)
                        nc.scalar.activation(pT[:, last0:], sc[:, last0:],
                                             EXP, bias=bias_sb[:, b:b + 1])
                        ops = out_ps_pool.tile([QSH, DV + 1], F32, tag="ops")
                        for c in range(kcb):
                            vt, m = vt_b[c]
                            nc.tensor.matmul(ops[:],
                                             pT[:m, c * QSH:(c + 1) * QSH],
                                             vt[:m, 0:DV + 1],
                                             start=(c == 0),
                                             stop=(c == kcb - 1))
                        state["ops"] = ops
                    return exp_final

                def make_div(b=b, state=state):
                    def div():
                        ops = state["ops"]
                        r = osb_pool.tile([QSH, 1], F32, tag="r")
                        nc.vector.reciprocal(r[:], ops[:, DV:DV + 1])
                        osb = osb_pool.tile([QSH, DV], F32, tag="osb")
                        nc.vector.tensor_scalar_mul(osb[:], ops[:, 0:DV], r[:])
                        nc.sync.dma_start(out_d[b, :, :], osb[:])
                    return div

                pend_exp = make_exp_final()
                div_q.append(make_div())
            pend_exp()
            for dv in div_q:
                dv()
    nc.compile()
    return nc


def _prep(queries, keys, values, valid_lens):
    vl = np.asarray(valid_lens).astype(np.int64)
    nps = tuple(_npad(int(l)) for l in vl)
    KT = sum(nps)

    kT = np.empty((D, KT), np.float32)
    vals = np.zeros((KT, DV + 2), ml_dtypes.bfloat16)
    biasT = np.zeros((128, B), np.float32)
    off = 0
    for b in range(B):
        n = nps[b]
        kT[:, off:off + n] = keys[b, :n, :].T
        vals[off:off + n, 0:DV] = values[b, :n, :].astype(ml_dtypes.bfloat16)
        vals[off:off + n, DV] = ml_dtypes.bfloat16(1.0)
        kcb = (n + 127) // 128
        j = np.arange(128)
        valid = (kcb - 1) * 128 + j < vl[b]
        biasT[:, b] = np.where(valid, 0.0, NEG_BIAS).astype(np.float32)
        off += n

    qT_shards = []
    for i in range(NCORES):
        qt = np.empty((D, B * QSH), np.float32)
        for b in range(B):
            qt[:, b * QSH:(b + 1) * QSH] = queries[b, i * QSH:(i + 1) * QSH, :].T
        qT_shards.append(qt)
    return nps, kT, vals, biasT, qT_shards


def run(queries, keys, values, valid_lens, W_q, W_k, w_v, **run_kwargs):
    """Full pipeline; returns (output, BassKernelResults)."""
    queries = np.asarray(queries, np.float32)
    keys = np.asarray(keys, np.float32)
    values = np.asarray(values, np.float32)
    W_q = np.asarray(W_q, np.float32)
    W_k = np.asarray(W_k, np.float32)
    w_v = np.asarray(w_v, np.float32)

    nps, kT, vals, biasT, qT_shards = _prep(queries, keys, values, valid_lens)
    wv = np.ascontiguousarray(w_v.reshape(H, 1)).astype(ml_dtypes.bfloat16)
    common = {
        "kT": np.ascontiguousarray(kT),
        "vals": np.ascontiguousarray(vals),
        "wq": np.ascontiguousarray(W_q),
        "wk": np.ascontiguousarray(W_k),
        "wv": wv,
        "biasT": np.ascontiguousarray(biasT),
    }
    in_maps = [dict(common, qT=np.ascontiguousarray(q)) for q in qT_shards]

    nc = _graph_cache.get(nps)
    if nc is None:
        nc = _build(nps)
        _graph_cache[nps] = nc
    res = run_bass_kernel_spmd(nc, in_maps, core_ids=list(range(NCORES)),
                               **run_kwargs)
    out = np.empty((B, Q, DV), np.float32)
    for i in range(NCORES):
        out[:, i * QSH:(i + 1) * QSH, :] = res.results[i]["out"]
    return out, res


def kernel(queries, keys, values, valid_lens, W_q, W_k, w_v):
    out, _ = run(queries, keys, values, valid_lens, W_q, W_k, w_v)
    return out


# revision 35
# speedup vs baseline: 1.0449x; 1.0211x over previous
"""Additive attention (Bahdanau) Trainium2 kernel, SPMD over 8 NeuronCores.

Reference computation (per batch b):
    q = queries @ W_q                    [Q, H]
    k = keys    @ W_k                    [K, H]
    scores[q,k] = sum_h w_v[h] * tanh(q[q,h] + k[k,h])
    attn = masked_softmax(scores, valid_len)   (keys >= valid_len masked out)
    out = attn @ values                  [Q, Dv]

Distribution: queries are sharded across the 8 cores (each core gets a
32-query slice of every batch); keys/values/weights are replicated. Each
core computes the same-shaped, perfectly load-balanced slice of the
output. Keys are truncated per batch to the valid length (rounded up to
an even count), which prunes the dominant tanh work for every core
equally. No collectives needed.

Device algorithm per (core, batch):
  - kprojT [h=128, n] = W_k^T @ keys^T      (keys pre-transposed on host)
  - qprojT [h=128, 32] = W_q^T @ q-slice^T
  - x = kprojT + qprojT[:, qi]   (tensor_scalar add, bf16, DVE/GPSIMD)
  - f = tanh(x)                  (ScalarE, fused over 8 queries per op)
  - scoresT[k, qi] = f_chunk^T @ w_v   (PE, N=1 matmuls into PSUM columns)
  - pT = exp(scoresT [+ mask bias on the boundary chunk])
        no max-subtraction needed: |scores| <= ||w_v||_1 ~ 9, exp can't
        overflow fp32; masked lanes get bias -30000 -> exp == 0 exactly.
  - out[qi, v] = (pT^T @ [V | 1]) , then divide by the appended ones-column
    row-sum (softmax normalization).
"""

import numpy as np
import ml_dtypes

import concourse.bass as bass
import concourse.tile as tile
import concourse.bacc as bacc
from concourse import mybir
from concourse.bass_utils import run_bass_kernel_spmd

BF16 = mybir.dt.bfloat16
F32 = mybir.dt.float32
TANH = mybir.ActivationFunctionType.Tanh
EXP = mybir.ActivationFunctionType.Exp

B, Q, K, D, H, DV = 8, 256, 1024, 256, 128, 128
NCORES = 8
QSH = Q // NCORES          # queries per core per batch
GQ = 16                    # queries fused per tanh op
NEG_BIAS = -30000.0        # exp(-30000) == 0.0 exactly in fp32

_graph_cache: dict = {}


def _npad(vl):
    """Per-batch key extent: valid length rounded up to even (DVE 4x mode)."""
    return int(min(max(2 * ((vl + 1) // 2), 2), K))


def _build(nps):
    """Build the SPMD graph. nps: tuple of per-batch (even) key extents."""
    nc = bacc.Bacc("TRN2", target_bir_lowering=False, debug=False,
                   num_devices=NCORES)
    KT = sum(nps)
    kcs = [(n + 127) // 128 for n in nps]
    # batch emission order: small first (fast pipeline start), small last
    # (short epilogue tail), big ones in the middle.
    # two smallish batches first (fast pipeline start while big DMAs
    # land), then the rest largest-first, ending on the smallest batch
    # (short epilogue tail).
    asc = sorted(range(B), key=lambda b: kcs[b])
    order = asc[0:2] + sorted(asc[2:], key=lambda b: -kcs[b])

    kT_d = nc.dram_tensor("kT", (D, KT), F32, kind="ExternalInput").ap()
    v_d = nc.dram_tensor("vals", (KT, DV + 2), BF16, kind="ExternalInput").ap()
    qT_d = nc.dram_tensor("qT", (D, B * QSH), F32, kind="ExternalInput").ap()
    wq_d = nc.dram_tensor("wq", (D, H), F32, kind="ExternalInput").ap()
    wk_d = nc.dram_tensor("wk", (D, H), F32, kind="ExternalInput").ap()
    wv_d = nc.dram_tensor("wv", (H, 1), BF16, kind="ExternalInput").ap()
    bias_d = nc.dram_tensor("biasT", (128, B), F32, kind="ExternalInput").ap()
    out_d = nc.dram_tensor("out", (B, QSH, DV), F32, kind="ExternalOutput").ap()

    offs = np.concatenate([[0], np.cumsum(nps)]).astype(int)

    with tile.TileContext(nc) as tc:
        with (
            tc.tile_pool(name="const", bufs=1) as const,
            tc.tile_pool(name="kt", bufs=4) as kt_pool,
            tc.tile_pool(name="kproj", bufs=4) as kproj_pool,
            tc.tile_pool(name="qproj", bufs=4) as qproj_pool,
            tc.tile_pool(name="vt", bufs=sum(kcs)) as vpool,
            tc.tile_pool(name="x", bufs=3) as xpool,
            tc.tile_pool(name="pT", bufs=2) as ppool,
            tc.tile_pool(name="osb", bufs=2) as osb_pool,
            tc.tile_pool(name="proj_ps", bufs=3, space="PSUM") as proj_ps,
            tc.tile_pool(name="sc_ps", bufs=2, space="PSUM") as sc_ps_pool,
            tc.tile_pool(name="out_ps", bufs=3, space="PSUM") as out_ps_pool,
        ):
            # ---- constants + first-batch DMA (gpsimd queue starts first) --
            b0 = order[0]
            dmad = {}

            def dma_a(b, first=False):
                n, off = nps[b], offs[b]
                eng = nc.gpsimd if first else nc.sync
                kT0 = kt_pool.tile([128, n], F32, tag="kT0")
                kT1 = kt_pool.tile([128, n], F32, tag="kT1")
                eng.dma_start(kT0[:], kT_d[0:128, off:off + n])
                eng.dma_start(kT1[:], kT_d[128:256, off:off + n])
                qt = kt_pool.tile([128, 2, QSH], F32, tag="qt")
                eng.dma_start(qt[:, 0, :], qT_d[0:128, b * QSH:(b + 1) * QSH])
                eng.dma_start(qt[:, 1, :], qT_d[128:256, b * QSH:(b + 1) * QSH])
                vt_b = []
                for c in range((n + 127) // 128):
                    m = min(128, n - c * 128)
                    vt = vpool.tile([128, DV + 2], BF16, tag="vt")
                    r0 = off + c * 128
                    nc.gpsimd.dma_start(vt[:m, :], v_d[r0:r0 + m, :])
                    vt_b.append((vt, m))
                dmad[b] = (kT0, kT1, qt, vt_b)

            dma_a(b0, first=True)

            wq_sb = const.tile([128, 2, H], F32, tag="wq")
            nc.sync.dma_start(wq_sb[:, 0, :], wq_d[0:128, :])
            nc.sync.dma_start(wq_sb[:, 1, :], wq_d[128:256, :])
            wk_sb = const.tile([128, 2, H], F32, tag="wk")
            nc.sync.dma_start(wk_sb[:, 0, :], wk_d[0:128, :])
            nc.sync.dma_start(wk_sb[:, 1, :], wk_d[128:256, :])
            wv_sb = const.tile([H, 1], BF16, tag="wv")
            nc.sync.dma_start(wv_sb[:], wv_d[:, :])
            bias_sb = const.tile([128, B], F32, tag="bias")
            nc.sync.dma_start(bias_sb[:], bias_d[:, :])

            projd = {}

            def proj_a(b):
                n = nps[b]
                kT0, kT1, qt, vt_b = dmad.pop(b)
                kp = kproj_pool.tile([128, n], BF16, tag="kp")
                for j0 in range(0, n, 512):
                    w = min(512, n - j0)
                    ps = proj_ps.tile([128, w], F32, tag="ps")
                    nc.tensor.matmul(ps[:], wk_sb[:, 0, :], kT0[:, j0:j0 + w],
                                     start=True, stop=False)
                    nc.tensor.matmul(ps[:], wk_sb[:, 1, :], kT1[:, j0:j0 + w],
                                     start=False, stop=True)
                    nc.vector.tensor_copy(kp[:, j0:j0 + w], ps[:])
                qp = qproj_pool.tile([128, QSH], F32, tag="qp")
                ps = proj_ps.tile([128, QSH], F32, tag="ps")
                nc.tensor.matmul(ps[:], wq_sb[:, 0, :], qt[:, 0, :],
                                 start=True, stop=False)
                nc.tensor.matmul(ps[:], wq_sb[:, 1, :], qt[:, 1, :],
                                 start=False, stop=True)
                nc.vector.tensor_copy(qp[:], ps[:])
                projd[b] = (kp, qp, vt_b)

            for bb in order[1:4]:
                dma_a(bb)
            proj_a(b0)

            # ---- per-batch pipeline ----
            # DMAs issued 2 batches ahead; projections + fp32->bf16 casts
            # 1 batch ahead (so the in-order DVE never stalls on a DMA);
            # exp+final-matmul of batch i-1 fire after batch i's first
            # score group; reciprocal/divide of batch i-1 (DVE, waits on
            # PE's final matmul) fire only after ALL of batch i's adds.
            pend_exp = None
            div_q = []
            for bi, b in enumerate(order):
                n = nps[b]
                kcb = kcs[b]
                m_last = n - (kcb - 1) * 128
                kprojT_b, qprojT_b, vt_b = projd.pop(b)
                sc = sc_ps_pool.tile([128, kcb * QSH], F32, tag="sc")
                if m_last < 128:
                    # kill stale PSUM rows in the partial chunk so
                    # exp(stale + bias) can't produce inf/nan; partition
                    # base must be 32-aligned, matmuls rewrite [0,m_last)
                    m0 = (m_last // 32) * 32
                    for p0 in range(m0, 128, 32):
                        nc.vector.memset(sc[p0:p0 + 32, (kcb - 1) * QSH:], 0.0)
                for g in range(QSH // GQ):
                    x = xpool.tile([128, GQ * n], BF16, tag="x")
                    for j in range(GQ):
                        qi = g * GQ + j
                        nc.vector.tensor_scalar_add(
                            x[:, j * n:(j + 1) * n], kprojT_b[:],
                            qprojT_b[:, qi:qi + 1])
                    nc.scalar.activation(x[:], x[:], TANH)  # in-place
                    for j in range(GQ):
                        qi = g * GQ + j
                        for c in range(kcb):
                            m = min(128, n - c * 128)
                            col = c * QSH + qi
                            nc.tensor.matmul(
                                sc[:m, col:col + 1],
                                x[:, j * n + c * 128:j * n + c * 128 + m],
                                wv_sb[:],
                                start=True, stop=True)
                    if g == 0:
                        if pend_exp is not None:
                            pend_exp()
                            pend_exp = None
                        if div_q and len(div_q) >= 2:
                            div_q.pop(0)()
                        if bi + 4 < B:
                            dma_a(order[bi + 4])
                        if bi + 1 < B:
                            proj_a(order[bi + 1])
                state = {}

                def make_exp_final(b=b, kcb=kcb, sc=sc, vt_b=vt_b,
                                   state=state):
                    def exp_final():
                        pT = ppool.tile([128, kcb * QSH], BF16, tag="pT")
                        last0 = (kcb - 1) * QSH
                        if kcb > 1:
                            nc.scalar.activation(pT[:, 0:last0],
                                                 sc[:, 0:last0], EXP)
                        nc.scalar.activation(pT[:, last0:], sc[:, last0:],
                                             EXP, bias=bias_sb[:, b:b + 1])
                        ops = out_ps_pool.tile([QSH, DV + 1], F32, tag="ops")
                        for c in range(kcb):
                            vt, m = vt_b[c]
                            nc.tensor.matmul(ops[:],
                                             pT[:m, c * QSH:(c + 1) * QSH],
                                             vt[:m, 0:DV + 1],
                                             start=(c == 0),
                                             stop=(c == kcb - 1))
                        state["ops"] = ops
                    return exp_final

                def make_div(b=b, state=state):
                    def div():
                        ops = state["ops"]
                        r = osb_pool.tile([QSH, 1], F32, tag="r")
                        nc.vector.reciprocal(r[:], ops[:, DV:DV + 1])
                        osb = osb_pool.tile([QSH, DV], F32, tag="osb")
                        nc.vector.tensor_scalar_mul(osb[:], ops[:, 0:DV], r[:])
                        nc.sync.dma_start(out_d[b, :, :], osb[:])
                    return div

                pend_exp = make_exp_final()
                div_q.append(make_div())
            pend_exp()
            for dv in div_q:
                dv()
    nc.compile()
    return nc


def _prep(queries, keys, values, valid_lens):
    vl = np.asarray(valid_lens).astype(np.int64)
    nps = tuple(_npad(int(l)) for l in vl)
    KT = sum(nps)

    kT = np.empty((D, KT), np.float32)
    vals = np.zeros((KT, DV + 2), ml_dtypes.bfloat16)
    biasT = np.zeros((128, B), np.float32)
    off = 0
    for b in range(B):
        n = nps[b]
        kT[:, off:off + n] = keys[b, :n, :].T
        vals[off:off + n, 0:DV] = values[b, :n, :].astype(ml_dtypes.bfloat16)
        vals[off:off + n, DV] = ml_dtypes.bfloat16(1.0)
        kcb = (n + 127) // 128
        j = np.arange(128)
        valid = (kcb - 1) * 128 + j < vl[b]
        biasT[:, b] = np.where(valid, 0.0, NEG_BIAS).astype(np.float32)
        off += n

    qT_shards = []
    for i in range(NCORES):
        qt = np.empty((D, B * QSH), np.float32)
        for b in range(B):
            qt[:, b * QSH:(b + 1) * QSH] = queries[b, i * QSH:(i + 1) * QSH, :].T
        qT_shards.append(qt)
    return nps, kT, vals, biasT, qT_shards


def run(queries, keys, values, valid_lens, W_q, W_k, w_v, **run_kwargs):
    """Full pipeline; returns (output, BassKernelResults)."""
    queries = np.asarray(queries, np.float32)
    keys = np.asarray(keys, np.float32)
    values = np.asarray(values, np.float32)
    W_q = np.asarray(W_q, np.float32)
    W_k = np.asarray(W_k, np.float32)
    w_v = np.asarray(w_v, np.float32)

    nps, kT, vals, biasT, qT_shards = _prep(queries, keys, values, valid_lens)
    wv = np.ascontiguousarray(w_v.reshape(H, 1)).astype(ml_dtypes.bfloat16)
    common = {
        "kT": np.ascontiguousarray(kT),
        "vals": np.ascontiguousarray(vals),
        "wq": np.ascontiguousarray(W_q),
        "wk": np.ascontiguousarray(W_k),
        "wv": wv,
        "biasT": np.ascontiguousarray(biasT),
    }
    in_maps = [dict(common, qT=np.ascontiguousarray(q)) for q in qT_shards]

    nc = _graph_cache.get(nps)
    if nc is None:
        nc = _build(nps)
        _graph_cache[nps] = nc
    res = run_bass_kernel_spmd(nc, in_maps, core_ids=list(range(NCORES)),
                               **run_kwargs)
    out = np.empty((B, Q, DV), np.float32)
    for i in range(NCORES):
        out[:, i * QSH:(i + 1) * QSH, :] = res.results[i]["out"]
    return out, res


def kernel(queries, keys, values, valid_lens, W_q, W_k, w_v):
    out, _ = run(queries, keys, values, valid_lens, W_q, W_k, w_v)
    return out


# revision 36
# speedup vs baseline: 1.0696x; 1.0236x over previous
"""Additive attention (Bahdanau) Trainium2 kernel, SPMD over 8 NeuronCores.

Reference computation (per batch b):
    q = queries @ W_q                    [Q, H]
    k = keys    @ W_k                    [K, H]
    scores[q,k] = sum_h w_v[h] * tanh(q[q,h] + k[k,h])
    attn = masked_softmax(scores, valid_len)   (keys >= valid_len masked out)
    out = attn @ values                  [Q, Dv]

Distribution: queries are sharded across the 8 cores (each core gets a
32-query slice of every batch); keys/values/weights are replicated. Each
core computes the same-shaped, perfectly load-balanced slice of the
output. Keys are truncated per batch to the valid length (rounded up to
an even count), which prunes the dominant tanh work for every core
equally. No collectives needed.

Device algorithm per (core, batch):
  - kprojT [h=128, n] = W_k^T @ keys^T      (keys pre-transposed on host)
  - qprojT [h=128, 32] = W_q^T @ q-slice^T
  - x = kprojT + qprojT[:, qi]   (tensor_scalar add, bf16, DVE/GPSIMD)
  - f = tanh(x)                  (ScalarE, fused over 8 queries per op)
  - scoresT[k, qi] = f_chunk^T @ w_v   (PE, N=1 matmuls into PSUM columns)
  - pT = exp(scoresT [+ mask bias on the boundary chunk])
        no max-subtraction needed: |scores| <= ||w_v||_1 ~ 9, exp can't
        overflow fp32; masked lanes get bias -30000 -> exp == 0 exactly.
  - out[qi, v] = (pT^T @ [V | 1]) , then divide by the appended ones-column
    row-sum (softmax normalization).
"""

import numpy as np
import ml_dtypes

import concourse.bass as bass
import concourse.tile as tile
import concourse.bacc as bacc
from concourse import mybir
from concourse.bass_utils import run_bass_kernel_spmd

BF16 = mybir.dt.bfloat16
F32 = mybir.dt.float32
TANH = mybir.ActivationFunctionType.Tanh
EXP = mybir.ActivationFunctionType.Exp

B, Q, K, D, H, DV = 8, 256, 1024, 256, 128, 128
NCORES = 8
QSH = Q // NCORES          # queries per core per batch
GQ = 16                    # queries fused per tanh op
NEG_BIAS = -30000.0        # exp(-30000) == 0.0 exactly in fp32

_graph_cache: dict = {}


def _npad(vl):
    """Per-batch key extent: valid length rounded up to even (DVE 4x mode)."""
    return int(min(max(2 * ((vl + 1) // 2), 2), K))


def _build(nps):
    """Build the SPMD graph. nps: tuple of per-batch (even) key extents."""
    nc = bacc.Bacc("TRN2", target_bir_lowering=False, debug=False,
                   num_devices=NCORES)
    KT = sum(nps)
    kcs = [(n + 127) // 128 for n in nps]
    # batch emission order: small first (fast pipeline start), small last
    # (short epilogue tail), big ones in the middle.
    # two smallish batches first (fast pipeline start while big DMAs
    # land), then the rest largest-first, ending on the smallest batch
    # (short epilogue tail).
    asc = sorted(range(B), key=lambda b: kcs[b])
    order = asc[0:2] + sorted(asc[2:], key=lambda b: -kcs[b])

    kT_d = nc.dram_tensor("kT", (D, KT), F32, kind="ExternalInput").ap()
    v_d = nc.dram_tensor("vals", (KT, DV + 2), BF16, kind="ExternalInput").ap()
    qT_d = nc.dram_tensor("qT", (D, B * QSH), F32, kind="ExternalInput").ap()
    wq_d = nc.dram_tensor("wq", (D, H), F32, kind="ExternalInput").ap()
    wk_d = nc.dram_tensor("wk", (D, H), F32, kind="ExternalInput").ap()
    wv_d = nc.dram_tensor("wv", (H, 1), BF16, kind="ExternalInput").ap()
    bias_d = nc.dram_tensor("biasT", (128, B), F32, kind="ExternalInput").ap()
    out_d = nc.dram_tensor("out", (B, QSH, DV), F32, kind="ExternalOutput").ap()

    offs = np.concatenate([[0], np.cumsum(nps)]).astype(int)

    with tile.TileContext(nc) as tc:
        with (
            tc.tile_pool(name="const", bufs=1) as const,
            tc.tile_pool(name="kt", bufs=4) as kt_pool,
            tc.tile_pool(name="kproj", bufs=4) as kproj_pool,
            tc.tile_pool(name="qproj", bufs=4) as qproj_pool,
            tc.tile_pool(name="vt", bufs=sum(kcs)) as vpool,
            tc.tile_pool(name="x", bufs=4) as xpool,
            tc.tile_pool(name="pT", bufs=2) as ppool,
            tc.tile_pool(name="osb", bufs=2) as osb_pool,
            tc.tile_pool(name="proj_ps", bufs=3, space="PSUM") as proj_ps,
            tc.tile_pool(name="sc_ps", bufs=2, space="PSUM") as sc_ps_pool,
            tc.tile_pool(name="out_ps", bufs=3, space="PSUM") as out_ps_pool,
        ):
            # ---- constants + first-batch DMA (gpsimd queue starts first) --
            b0 = order[0]
            dmad = {}

            def dma_a(b, first=False):
                n, off = nps[b], offs[b]
                eng = nc.gpsimd if first else nc.sync
                kT0 = kt_pool.tile([128, n], F32, tag="kT0")
                kT1 = kt_pool.tile([128, n], F32, tag="kT1")
                eng.dma_start(kT0[:], kT_d[0:128, off:off + n])
                eng.dma_start(kT1[:], kT_d[128:256, off:off + n])
                qt = kt_pool.tile([128, 2, QSH], F32, tag="qt")
                eng.dma_start(qt[:, 0, :], qT_d[0:128, b * QSH:(b + 1) * QSH])
                eng.dma_start(qt[:, 1, :], qT_d[128:256, b * QSH:(b + 1) * QSH])
                vt_b = []
                for c in range((n + 127) // 128):
                    m = min(128, n - c * 128)
                    vt = vpool.tile([128, DV + 2], BF16, tag="vt")
                    r0 = off + c * 128
                    nc.gpsimd.dma_start(vt[:m, :], v_d[r0:r0 + m, :])
                    vt_b.append((vt, m))
                dmad[b] = (kT0, kT1, qt, vt_b)

            dma_a(b0, first=True)

            wq_sb = const.tile([128, 2, H], F32, tag="wq")
            nc.sync.dma_start(wq_sb[:, 0, :], wq_d[0:128, :])
            nc.sync.dma_start(wq_sb[:, 1, :], wq_d[128:256, :])
            wk_sb = const.tile([128, 2, H], F32, tag="wk")
            nc.sync.dma_start(wk_sb[:, 0, :], wk_d[0:128, :])
            nc.sync.dma_start(wk_sb[:, 1, :], wk_d[128:256, :])
            wv_sb = const.tile([H, 1], BF16, tag="wv")
            nc.sync.dma_start(wv_sb[:], wv_d[:, :])
            bias_sb = const.tile([128, B], F32, tag="bias")
            nc.sync.dma_start(bias_sb[:], bias_d[:, :])

            projd = {}

            def proj_a(b):
                n = nps[b]
                kT0, kT1, qt, vt_b = dmad.pop(b)
                kp = kproj_pool.tile([128, n], BF16, tag="kp")
                for j0 in range(0, n, 512):
                    w = min(512, n - j0)
                    ps = proj_ps.tile([128, w], F32, tag="ps")
                    nc.tensor.matmul(ps[:], wk_sb[:, 0, :], kT0[:, j0:j0 + w],
                                     start=True, stop=False)
                    nc.tensor.matmul(ps[:], wk_sb[:, 1, :], kT1[:, j0:j0 + w],
                                     start=False, stop=True)
                    nc.vector.tensor_copy(kp[:, j0:j0 + w], ps[:])
                qp = qproj_pool.tile([128, QSH], F32, tag="qp")
                ps = proj_ps.tile([128, QSH], F32, tag="ps")
                nc.tensor.matmul(ps[:], wq_sb[:, 0, :], qt[:, 0, :],
                                 start=True, stop=False)
                nc.tensor.matmul(ps[:], wq_sb[:, 1, :], qt[:, 1, :],
                                 start=False, stop=True)
                nc.vector.tensor_copy(qp[:], ps[:])
                projd[b] = (kp, qp, vt_b)

            for bb in order[1:4]:
                dma_a(bb)
            proj_a(b0)

            # ---- per-batch pipeline ----
            # DMAs issued 2 batches ahead; projections + fp32->bf16 casts
            # 1 batch ahead (so the in-order DVE never stalls on a DMA);
            # exp+final-matmul of batch i-1 fire after batch i's first
            # score group; reciprocal/divide of batch i-1 (DVE, waits on
            # PE's final matmul) fire only after ALL of batch i's adds.
            pend_exp = None
            div_q = []
            for bi, b in enumerate(order):
                n = nps[b]
                kcb = kcs[b]
                m_last = n - (kcb - 1) * 128
                kprojT_b, qprojT_b, vt_b = projd.pop(b)
                sc = sc_ps_pool.tile([128, kcb * QSH], F32, tag="sc")
                if m_last < 128:
                    # kill stale PSUM rows in the partial chunk so
                    # exp(stale + bias) can't produce inf/nan; partition
                    # base must be 32-aligned, matmuls rewrite [0,m_last)
                    m0 = (m_last // 32) * 32
                    for p0 in range(m0, 128, 32):
                        nc.vector.memset(sc[p0:p0 + 32, (kcb - 1) * QSH:], 0.0)
                for g in range(QSH // GQ):
                    x = xpool.tile([128, GQ * n], BF16, tag="x")
                    for j in range(GQ):
                        qi = g * GQ + j
                        nc.vector.tensor_scalar_add(
                            x[:, j * n:(j + 1) * n], kprojT_b[:],
                            qprojT_b[:, qi:qi + 1])
                    nc.scalar.activation(x[:], x[:], TANH)  # in-place
                    for j in range(GQ):
                        qi = g * GQ + j
                        for c in range(kcb):
                            m = min(128, n - c * 128)
                            col = c * QSH + qi
                            nc.tensor.matmul(
                                sc[:m, col:col + 1],
                                x[:, j * n + c * 128:j * n + c * 128 + m],
                                wv_sb[:],
                                start=True, stop=True)
                    if g == 0:
                        if pend_exp is not None:
                            pend_exp()
                            pend_exp = None
                        if div_q and len(div_q) >= 2:
                            div_q.pop(0)()
                        if bi + 4 < B:
                            dma_a(order[bi + 4])
                        if bi + 1 < B:
                            proj_a(order[bi + 1])
                state = {}

                def make_exp_final(b=b, kcb=kcb, sc=sc, vt_b=vt_b,
                                   state=state):
                    def exp_final():
                        pT = ppool.tile([128, kcb * QSH], BF16, tag="pT")
                        last0 = (kcb - 1) * QSH
                        if kcb > 1:
                            nc.scalar.activation(pT[:, 0:last0],
                                                 sc[:, 0:last0], EXP)
                        nc.scalar.activation(pT[:, last0:], sc[:, last0:],
                                             EXP, bias=bias_sb[:, b:b + 1])
                        ops = out_ps_pool.tile([QSH, DV + 1], F32, tag="ops")
                        for c in range(kcb):
                            vt, m = vt_b[c]
                            nc.tensor.matmul(ops[:],
                                             pT[:m, c * QSH:(c + 1) * QSH],
                                             vt[:m, 0:DV + 1],
                                             start=(c == 0),
                                             stop=(c == kcb - 1))
                        state["ops"] = ops
                    return exp_final

                def make_div(b=b, state=state):
                    def div():
                        ops = state["ops"]
                        r = osb_pool.tile([QSH, 1], F32, tag="r")
                        nc.vector.reciprocal(r[:], ops[:, DV:DV + 1])
                        osb = osb_pool.tile([QSH, DV], F32, tag="osb")
                        nc.vector.tensor_scalar_mul(osb[:], ops[:, 0:DV], r[:])
                        nc.sync.dma_start(out_d[b, :, :], osb[:])
                    return div

                pend_exp = make_exp_final()
                div_q.append(make_div())
            pend_exp()
            for dv in div_q:
                dv()
    nc.compile()
    return nc


def _prep(queries, keys, values, valid_lens):
    vl = np.asarray(valid_lens).astype(np.int64)
    nps = tuple(_npad(int(l)) for l in vl)
    KT = sum(nps)

    kT = np.empty((D, KT), np.float32)
    vals = np.zeros((KT, DV + 2), ml_dtypes.bfloat16)
    biasT = np.zeros((128, B), np.float32)
    off = 0
    for b in range(B):
        n = nps[b]
        kT[:, off:off + n] = keys[b, :n, :].T
        vals[off:off + n, 0:DV] = values[b, :n, :].astype(ml_dtypes.bfloat16)
        vals[off:off + n, DV] = ml_dtypes.bfloat16(1.0)
        kcb = (n + 127) // 128
        j = np.arange(128)
        valid = (kcb - 1) * 128 + j < vl[b]
        biasT[:, b] = np.where(valid, 0.0, NEG_BIAS).astype(np.float32)
        off += n

    qT_shards = []
    for i in range(NCORES):
        qt = np.empty((D, B * QSH), np.float32)
        for b in range(B):
            qt[:, b * QSH:(b + 1) * QSH] = queries[b, i * QSH:(i + 1) * QSH, :].T
        qT_shards.append(qt)
    return nps, kT, vals, biasT, qT_shards


def run(queries, keys, values, valid_lens, W_q, W_k, w_v, **run_kwargs):
    """Full pipeline; returns (output, BassKernelResults)."""
    queries = np.asarray(queries, np.float32)
    keys = np.asarray(keys, np.float32)
    values = np.asarray(values, np.float32)
    W_q = np.asarray(W_q, np.float32)
    W_k = np.asarray(W_k, np.float32)
    w_v = np.asarray(w_v, np.float32)

    nps, kT, vals, biasT, qT_shards = _prep(queries, keys, values, valid_lens)
    wv = np.ascontiguousarray(w_v.reshape(H, 1)).astype(ml_dtypes.bfloat16)
    common = {
        "kT": np.ascontiguousarray(kT),
        "vals": np.ascontiguousarray(vals),
        "wq": np.ascontiguousarray(W_q),
        "wk": np.ascontiguousarray(W_k),
        "wv": wv,
        "biasT": np.ascontiguousarray(biasT),
    }
    in_maps = [dict(common, qT=np.ascontiguousarray(q)) for q in qT_shards]

    nc = _graph_cache.get(nps)
    if nc is None:
        nc = _build(nps)
        _graph_cache[nps] = nc
    res = run_bass_kernel_spmd(nc, in_maps, core_ids=list(range(NCORES)),
                               **run_kwargs)
    out = np.empty((B, Q, DV), np.float32)
    for i in range(NCORES):
        out[:, i * QSH:(i + 1) * QSH, :] = res.results[i]["out"]
    return out, res


def kernel(queries, keys, values, valid_lens, W_q, W_k, w_v):
    out, _ = run(queries, keys, values, valid_lens, W_q, W_k, w_v)
    return out
